# revision 2
# baseline (speedup 1.0000x reference)
"""Trainium2 Bass kernel for nn_CustomCIFAR10Model.

Math (reference):
    xf = x.reshape(B, D)
    part2[b,d] = cos(xf[b,d]) * Sa[d] + sin(xf[b,d]) * Sb[d]
        where Sa[d] = sum_i a[i,d,0], Sb[d] = sum_i b[i,d,0]
    part1 = sum(w[1:]*n[1:] + w[:-1]*n[:-1])            (scalar)
    out = (part1 + part2) @ fc_w.T + fc_b               [B, NCLS]

Memory-bound: the only heavy work is streaming a and b once to
column-sum them. Sharding: d-columns split across 8 cores (384 each);
every core reduces its a/b slice, builds cos*Sa + sin*Sb implicitly by
scaling fc_w columns, and contracts to a partial [NCLS, B] output; the
host sums the 8 partials and adds part1/bias.

Key changes vs the f32 baseline (45-51 us):
 - a/b/x are cast to fp16 on the host: the dominant DMA stream halves
   to ~5.1 MB/core. Error budget: fp16 rounding of a adds ~3e-4
   relative error to Sa (sqrt(3072)-averaged), far under the 2e-2 gate.
 - All tiles are SBUF-resident (no pool recycling), so every load DMA
   is issued back-to-back at kernel start; pieces are tapered (big
   first, 1-chunk last) so the PE consumes while streaming and the
   last-byte -> output dependency chain is short.
 - Loads are spread over both HWDGE rings (sync + scalar) so the two
   FIFO rings interleave; fwt rides the gpsimd (SWDGE) ring.
 - Trig is spread across ACT (copy+sin), DVE (cos shift + cos magic
   round), and GpSimd (sin magic round) so no single engine's
   elementwise work exceeds the DMA stream time.
 - Dummy matmuls at kernel start keep the PE HAM clock warm.
 - Output store: PSUM->SBUF copy split by columns across ACT/DVE, then
   two partition-contiguous DMAs on the two HWDGE rings.

HW Sin only accepts [-pi, pi]: range-reduce t = x/(2pi), r = t - round(t)
via the fp32 magic-number trick, then Sin(2pi*r); cos shifts t by +1/4.
"""

import numpy as np

B = 512
D = 3072
NCLS = 100
P = 128
NCORES = 8
DW = D // NCORES          # 384 columns per core
NSUB = DW // P            # 3 d-subtiles of 128
NCH = D // P              # 24 row-chunks of the a/b slice

# Load piece boundaries (in chunks) per ring. Tapered: the last pieces
# are small so the tail after the final byte is short.
A_PIECES_SYNC = [(0, 4), (4, 12), (12, 24)]
B_PIECES_SCALAR = [(0, 8), (8, 16), (16, 21)]
B_PIECES_SYNC = [(21, 23), (23, 24)]

_STATE = {}


def _build():
    """Build + bacc-compile the SPMD Bass program (once per process)."""
    import concourse.bacc as bacc
    import concourse.mybir as mybir
    import concourse.tile as tile

    f32 = mybir.dt.float32
    f32r = mybir.dt.float32r
    f16 = mybir.dt.float16
    nc = bacc.Bacc(
        "TRN2", target_bir_lowering=False, debug=False, num_devices=NCORES
    )

    a_s = nc.dram_tensor("a_s", [P, NCH * DW], f16, kind="ExternalInput")
    b_s = nc.dram_tensor("b_s", [P, NCH * DW], f16, kind="ExternalInput")
    xt_s = nc.dram_tensor("xt_s", [DW, B], f16, kind="ExternalInput")
    fwt_s = nc.dram_tensor("fwt_s", [DW, NCLS], f32r, kind="ExternalInput")
    out_cb = nc.dram_tensor("out_cb", [NCLS, B], f32, kind="ExternalOutput")

    INV2PI = float(1.0 / (2.0 * np.pi))
    TWO_PI = float(2.0 * np.pi)
    MAGIC = float(1.5 * 2.0**23)
    add_op = mybir.AluOpType.add
    sub_op = mybir.AluOpType.subtract
    Sin = mybir.ActivationFunctionType.Sin
    Copy = mybir.ActivationFunctionType.Copy

    with tile.TileContext(nc) as tc:
        with (
            tc.tile_pool(name="consts", bufs=1) as const_pool,
            tc.tile_pool(name="xwork", bufs=1) as x_pool,
            tc.tile_pool(name="ps", bufs=2, space="PSUM") as psum_pool,
            tc.tile_pool(name="psrow", bufs=1, space="PSUM") as psum_row_pool,
            tc.tile_pool(name="psout", bufs=1, space="PSUM") as psum_out_pool,
            tc.tile_pool(name="pswarm", bufs=1, space="PSUM") as psum_warm_pool,
        ):
            ones_h = const_pool.tile([P, 1], f16, name="ones_h")
            nc.vector.memset(ones_h[:], 1.0)
            one1 = const_pool.tile([1, 1], f32, name="one1")
            nc.vector.memset(one1[:], 1.0)
            zero = const_pool.tile([P, 1], f32, name="zerob")
            nc.vector.memset(zero[:], 0.0)
            warmz = const_pool.tile([P, DW], f16, name="warmz")
            nc.vector.memset(warmz[:], 0.0)

            # ---------- load DMAs, all issued up front ----------
            a_sb = x_pool.tile([P, NCH, DW], f16, name="a_sb")
            b_sb = x_pool.tile([P, NCH, DW], f16, name="b_sb")
            xt = x_pool.tile([P, NSUB, B], f16, name="xt")
            fwt = x_pool.tile([P, NSUB, NCLS], f32r, name="fwt")

            for (c0, c1) in A_PIECES_SYNC:
                nc.sync.dma_start(
                    out=a_sb[:, c0:c1, :], in_=a_s[:, c0 * DW : c1 * DW]
                )
            for (c0, c1) in B_PIECES_SYNC:
                nc.sync.dma_start(
                    out=b_sb[:, c0:c1, :], in_=b_s[:, c0 * DW : c1 * DW]
                )
            nc.scalar.dma_start(
                out=xt[:], in_=xt_s[:].rearrange("(s p) b -> p s b", p=P)
            )
            for (c0, c1) in B_PIECES_SCALAR:
                nc.scalar.dma_start(
                    out=b_sb[:, c0:c1, :], in_=b_s[:, c0 * DW : c1 * DW]
                )
            nc.gpsimd.dma_start(
                out=fwt[:], in_=fwt_s[:].rearrange("(s p) c -> p s c", p=P)
            )

            # Dummy Sin so the Sin table set loads once at kernel start;
            # Copy is a filler in every set, so later Copy ACTIVATEs
            # reuse the resident set (no reload).
            warm = const_pool.tile([P, 1], f32, name="warm")
            nc.scalar.activation(warm[:], zero[:], Sin, bias=zero[:])

            # PE warm-up: data-independent matmuls right after the
            # preamble keep the HAM activity window busy so the real
            # reduction runs at 2.4 GHz.
            warm_ps = psum_warm_pool.tile([1, DW], f32, name="warm_ps")
            for _ in range(8):
                nc.tensor.matmul(
                    warm_ps[:], ones_h[:], warmz[:], start=True, stop=True
                )

            # ---------- column-sum reduction (PE) ----------
            rows = []
            for ti in range(2):
                psr = psum_row_pool.tile([1, DW], f32, name=f"psr{ti}", tag=f"psr{ti}")
                rows.append(psr)
            srcs = (a_sb, b_sb)
            emitted = [0, 0]

            def chunk_mms(ti, c0, c1):
                for c in range(c0, c1):
                    nc.tensor.matmul(
                        rows[ti][:],
                        ones_h[:],
                        srcs[ti][:, c, :],
                        start=(emitted[ti] == 0),
                        stop=(emitted[ti] == NCH - 1),
                    )
                    emitted[ti] += 1

            for (c0, c1) in A_PIECES_SYNC:
                chunk_mms(0, c0, c1)

            # ---------- trig on x (overlaps the stream) ----------
            # sin side: ts = x/(2pi)        [ACT], magic-round on GpSimd
            # cos side: tc = ts + 1/4       [DVE], magic-round on DVE
            sins = []
            coss = []
            for sub in range(NSUB):
                xts = xt[:, sub, :]
                ts_t = x_pool.tile([P, B], f32, name=f"ts{sub}")
                nc.scalar.activation(ts_t[:], xts, Copy, bias=0.0, scale=INV2PI)
                tc_t = x_pool.tile([P, B], f32, name=f"tc{sub}")
                nc.vector.tensor_scalar_add(tc_t[:], ts_t[:], 0.25)
                kc_t = x_pool.tile([P, B], f32, name=f"kc{sub}")
                nc.vector.tensor_scalar(kc_t[:], tc_t[:], MAGIC, MAGIC, add_op, sub_op)
                rc_t = x_pool.tile([P, B], f32, name=f"rc{sub}")
                nc.vector.tensor_sub(rc_t[:], tc_t[:], kc_t[:])
                cosv = x_pool.tile([P, B], f32r, name=f"cos{sub}")
                nc.scalar.activation(cosv[:], rc_t[:], Sin, bias=zero[:], scale=TWO_PI)
                coss.append(cosv)
                ks_t = x_pool.tile([P, B], f32, name=f"ks{sub}")
                nc.gpsimd.tensor_scalar(ks_t[:], ts_t[:], MAGIC, MAGIC, add_op, sub_op)
                rs_t = x_pool.tile([P, B], f32, name=f"rs{sub}")
                nc.gpsimd.tensor_sub(rs_t[:], ts_t[:], ks_t[:])
                sinv = x_pool.tile([P, B], f32r, name=f"sin{sub}")
                nc.scalar.activation(sinv[:], rs_t[:], Sin, bias=zero[:], scale=TWO_PI)
                sins.append(sinv)

            out_ps = psum_out_pool.tile([NCLS, B], f32, name="out_ps")

            def finish_tensor(ti, vals, start):
                """Transpose row ti to per-partition cols, scale the
                SMALL fwt tiles by them (fwt[d,c]*S[d]), and accumulate
                (fwt*S).T @ trig into out_ps."""
                for sub in range(NSUB):
                    row_sb = const_pool.tile(
                        [1, P], f32, name=f"row{ti}_{sub}", tag=f"row{ti}_{sub}"
                    )
                    nc.vector.tensor_copy(
                        row_sb[:], rows[ti][0:1, sub * P : (sub + 1) * P]
                    )
                    ps = psum_pool.tile([P, 1], f32, name=f"ps{ti}_{sub}", tag="ps")
                    nc.tensor.matmul(
                        ps[:], row_sb[:], one1[:], start=True, stop=True
                    )
                    fws = x_pool.tile(
                        [P, NCLS], f32r, name=f"fws{ti}_{sub}", tag=f"fws{ti}{sub}"
                    )
                    nc.vector.tensor_scalar_mul(fws[:], fwt[:, sub, :], ps[:])
                    nc.tensor.matmul(
                        out_ps[:],
                        fws[:],
                        vals[sub][:],
                        start=(start and sub == 0),
                        stop=(not start and sub == NSUB - 1),
                    )

            # a finishes mid-stream: its cos-side output matmuls overlap
            # the b stream; b's sin side forms the (short) tail.
            finish_tensor(0, coss, start=True)
            for (c0, c1) in B_PIECES_SCALAR:
                chunk_mms(1, c0, c1)
            for (c0, c1) in B_PIECES_SYNC:
                chunk_mms(1, c0, c1)
            finish_tensor(1, sins, start=False)

            # ---------- output store ----------
            # Column-split PSUM->SBUF copies on two engines, then two
            # partition-contiguous DMAs on the two HWDGE rings.
            out_sb = const_pool.tile([NCLS, B], f32, name="out_sb")
            H = B // 2
            nc.scalar.copy(out_sb[:, 0:H], out_ps[:, 0:H])
            nc.vector.tensor_copy(out_sb[:, H:B], out_ps[:, H:B])
            PH = NCLS // 2
            nc.sync.dma_start(out=out_cb[0:PH, :], in_=out_sb[0:PH, :])
            nc.scalar.dma_start(out=out_cb[PH:NCLS, :], in_=out_sb[PH:NCLS, :])

    nc.compile()
    return nc


def _get_nc():
    if "nc" not in _STATE:
        _STATE["nc"] = _build()
    return _STATE["nc"]


def _pack_ab(t2, sl):
    """[D, DW] f32 slice -> [P, NCH*DW] fp16, chunk-major free dim."""
    s = t2[:, sl].reshape(NCH, P, DW).transpose(1, 0, 2).reshape(P, NCH * DW)
    return np.ascontiguousarray(s.astype(np.float16))


def _prep_in_maps(x, a, b, fc_w):
    xf = np.asarray(x, dtype=np.float32).reshape(B, D)
    xt = np.ascontiguousarray(xf.T.astype(np.float16))  # [D, B] fp16
    a2 = np.asarray(a, dtype=np.float32).reshape(D, D)
    b2 = np.asarray(b, dtype=np.float32).reshape(D, D)
    fw = np.asarray(fc_w, dtype=np.float32)
    in_maps = []
    for m in range(NCORES):
        sl = slice(m * DW, (m + 1) * DW)
        in_maps.append(
            {
                "a_s": _pack_ab(a2, sl),
                "b_s": _pack_ab(b2, sl),
                "xt_s": np.ascontiguousarray(xt[sl, :]),
                "fwt_s": np.ascontiguousarray(fw[:, sl].T),
            }
        )
    return in_maps


def _run(inputs, trace=False, trace_kwargs=None):
    """Run the device kernel; returns (final_output, BassKernelResults)."""
    from concourse.bass_utils import run_bass_kernel_spmd

    x = inputs["x"]
    a = inputs["a"]
    b = inputs["b"]
    w = np.asarray(inputs["w"], dtype=np.float64)
    n_param = np.asarray(inputs["n_param"], dtype=np.float64)
    fc_w = np.asarray(inputs["fc_w"], dtype=np.float32)
    fc_b = np.asarray(inputs["fc_b"], dtype=np.float32)

    nc = _get_nc()
    in_maps = _prep_in_maps(x, a, b, fc_w)
    res = run_bass_kernel_spmd(
        nc,
        in_maps,
        list(range(NCORES)),
        trace=trace,
        **(trace_kwargs or {}),
    )

    acc = np.zeros((NCLS, B), dtype=np.float32)
    for r in res.results:
        acc += r["out_cb"]
    part1 = float(np.sum(w[1:] * n_param[1:] + w[:-1] * n_param[:-1]))
    final = acc.T + np.float32(part1) * fc_w.sum(axis=1)[None, :] + fc_b[None, :]
    return np.ascontiguousarray(final.astype(np.float32)), res


def kernel(**inputs) -> np.ndarray:
    out, _ = _run(inputs, trace=False)
    return out


# revision 4
# speedup vs baseline: 1.8046x; 1.8046x over previous
"""Trainium2 Bass kernel for nn_CustomCIFAR10Model.

Math (reference):
    xf = x.reshape(B, D)
    part2[b,d] = cos(xf[b,d]) * Sa[d] + sin(xf[b,d]) * Sb[d]
        where Sa[d] = sum_i a[i,d,0], Sb[d] = sum_i b[i,d,0]
    part1 = sum(w[1:]*n[1:] + w[:-1]*n[:-1])            (scalar)
    out = (part1 + part2) @ fc_w.T + fc_b               [B, NCLS]

Memory-bound: the only heavy work is streaming a and b once to
column-sum them. Sharding: d-columns split across 8 cores (384 each);
every core reduces its a/b slice, scales its fc_w columns by the sums,
and contracts against cos/sin(x) to a partial [NCLS, B] output; the
host sums the 8 partials and adds part1/bias.

Schedule (from trace analysis of prior versions):
 - a/b/x/fc_w cast to fp16 on the host: the stream halves to ~5.1 MB
   per core (adds ~3e-4 relative error, gate is 2e-2).
 - All tiles SBUF-resident; all load DMAs issue back-to-back at kernel
   start over both HWDGE rings, which drain evenly at ~200 GB/s each.
   a pieces are front-loaded on both rings so the cos-side finish
   overlaps the b stream; the last pieces are small so the tail after
   the final byte is short.  Each SBUF tile is written by exactly ONE
   ring: cross-queue writes to the same tile serialize on a hazard sem
   (cost ~4.5 us in v2), hence the _lo/_hi tile split.
 - Elementwise trig runs on DVE in fp16 (2x mode) + Sin on ACT.  The
   magic-round constant stays 1.5*2^23: DVE computes INTERNALLY in
   fp32, so the fused (t+M)-M tensor_scalar rounds exactly and only
   the small integer result lands in fp16.  (A fp16-scaled magic of
   1.5*2^10 does NOT work - the fp32 internal add keeps the fraction.)
   GpSimd runs nothing: its tensor ops measure ~20x slower than DVE.
 - 14 dummy matmuls bridge the PE from preamble end to first data so
   the HAM clock-gate reaches 2.4 GHz before the real reduction
   (fp16/f32r matmuls are single-pass; a cold PE paces at 320ns per
   384-col chunk vs 160ns warm).
 - Output store: PSUM->SBUF copy split by columns across ACT/DVE, then
   two partition-contiguous DMAs on the two HWDGE rings.

HW Sin only accepts [-pi, pi]: range-reduce t = x/(2pi), r = t - round(t)
via the magic trick, then Sin(2pi*r); cos shifts t by +1/4 first.
"""

import numpy as np

B = 512
D = 3072
NCLS = 100
P = 128
NCORES = 8
DW = D // NCORES          # 384 columns per core
NSUB = DW // P            # 3 d-subtiles of 128
NCH = D // P              # 24 row-chunks of the a/b slice

# Tile split points (chunks): tiles are single-ring so rings never
# co-write a tile.  sync ring: combo + a_lo + b_hi; scalar: a_hi + b_lo.
ALO = 10   # a chunks 0..ALO-1 on sync
BLO = 13   # b chunks 0..BLO-1 on scalar
A_LO_PIECES = [(0, 5), (5, 10)]
A_HI_PIECES = [(0, 7), (7, 14)]          # chunks 10..23
B_LO_PIECES = [(0, 7), (7, 11), (11, 13)]
B_HI_PIECES = [(0, 5), (5, 9), (9, 11)]  # chunks 13..23

_STATE = {}


def _build():
    """Build + bacc-compile the SPMD Bass program (once per process)."""
    import concourse.bacc as bacc
    import concourse.mybir as mybir
    import concourse.tile as tile

    f32 = mybir.dt.float32
    f16 = mybir.dt.float16
    nc = bacc.Bacc(
        "TRN2", target_bir_lowering=False, debug=False, num_devices=NCORES
    )

    a_s = nc.dram_tensor("a_s", [P, NCH * DW], f16, kind="ExternalInput")
    b_s = nc.dram_tensor("b_s", [P, NCH * DW], f16, kind="ExternalInput")
    # combo: per partition p, per sub s: [ x (B) | fwt (NCLS) ]
    cmb_s = nc.dram_tensor("cmb_s", [P, NSUB * (B + NCLS)], f16, kind="ExternalInput")
    out_cb = nc.dram_tensor("out_cb", [NCLS, B], f32, kind="ExternalOutput")

    INV2PI = float(1.0 / (2.0 * np.pi))
    TWO_PI = float(2.0 * np.pi)
    MAGIC = float(1.5 * 2.0**23)
    add_op = mybir.AluOpType.add
    sub_op = mybir.AluOpType.subtract
    Sin = mybir.ActivationFunctionType.Sin

    with tile.TileContext(nc) as tc:
        with (
            tc.tile_pool(name="consts", bufs=1) as const_pool,
            tc.tile_pool(name="xwork", bufs=1) as x_pool,
            tc.tile_pool(name="ps", bufs=2, space="PSUM") as psum_pool,
            tc.tile_pool(name="psrow", bufs=1, space="PSUM") as psum_row_pool,
            tc.tile_pool(name="psout", bufs=1, space="PSUM") as psum_out_pool,
            tc.tile_pool(name="pswarm", bufs=1, space="PSUM") as psum_warm_pool,
        ):
            ones_h = const_pool.tile([P, 1], f16, name="ones_h")
            nc.vector.memset(ones_h[:], 1.0)
            one1 = const_pool.tile([1, 1], f32, name="one1")
            nc.vector.memset(one1[:], 1.0)
            zero = const_pool.tile([P, 1], f32, name="zerob")
            nc.vector.memset(zero[:], 0.0)
            zero_h = const_pool.tile([P, 1], f16, name="zero_h")
            nc.vector.memset(zero_h[:], 0.0)
            warmz = const_pool.tile([P, DW], f16, name="warmz")
            nc.vector.memset(warmz[:], 0.0)

            # ---------- load DMAs, all issued up front ----------
            a_lo = x_pool.tile([P, ALO, DW], f16, name="a_lo")
            a_hi = x_pool.tile([P, NCH - ALO, DW], f16, name="a_hi")
            b_lo = x_pool.tile([P, BLO, DW], f16, name="b_lo")
            b_hi = x_pool.tile([P, NCH - BLO, DW], f16, name="b_hi")
            cmb = x_pool.tile([P, NSUB, B + NCLS], f16, name="cmb")

            def ld(eng, dst, src, base, c0, c1):
                eng.dma_start(
                    out=dst[:, c0:c1, :],
                    in_=src[:, (base + c0) * DW : (base + c1) * DW],
                )

            nc.sync.dma_start(
                out=cmb[:], in_=cmb_s[:].rearrange("p (s c) -> p s c", s=NSUB)
            )
            for (c0, c1) in A_LO_PIECES:
                ld(nc.sync, a_lo, a_s, 0, c0, c1)
            for (c0, c1) in B_HI_PIECES:
                ld(nc.sync, b_hi, b_s, BLO, c0, c1)
            for (c0, c1) in A_HI_PIECES:
                ld(nc.scalar, a_hi, a_s, ALO, c0, c1)
            for (c0, c1) in B_LO_PIECES:
                ld(nc.scalar, b_lo, b_s, 0, c0, c1)

            # Dummy Sin so the Sin table set loads once at kernel start.
            warm = const_pool.tile([P, 1], f32, name="warm")
            nc.scalar.activation(warm[:], zero[:], Sin, bias=zero[:])

            # PE warm-up bridge: preamble end -> first data arrival.
            warm_ps = psum_warm_pool.tile([1, DW], f32, name="warm_ps")
            for _ in range(14):
                nc.tensor.matmul(
                    warm_ps[:], ones_h[:], warmz[:], start=True, stop=True
                )

            # ---------- column-sum reduction (PE) ----------
            rows = []
            for ti in range(2):
                psr = psum_row_pool.tile([1, DW], f32, name=f"psr{ti}", tag=f"psr{ti}")
                rows.append(psr)
            emitted = [0, 0]

            def chunk_mms(ti, src, c0, c1):
                for c in range(c0, c1):
                    nc.tensor.matmul(
                        rows[ti][:],
                        ones_h[:],
                        src[:, c, :],
                        start=(emitted[ti] == 0),
                        stop=(emitted[ti] == NCH - 1),
                    )
                    emitted[ti] += 1

            # a chunks in approximate ring-arrival order
            chunk_mms(0, a_hi, 0, 7)
            chunk_mms(0, a_lo, 0, 5)
            chunk_mms(0, a_hi, 7, 14)
            chunk_mms(0, a_lo, 5, 10)

            # ---------- trig on x: fp16 on DVE + ACT ----------
            xt = cmb[:, :, 0:B]
            fwt = cmb[:, :, B : B + NCLS]
            sins = []
            coss = []
            for sub in range(NSUB):
                xts = xt[:, sub, :]
                ts_t = x_pool.tile([P, B], f16, name=f"ts{sub}")
                nc.vector.tensor_scalar_mul(ts_t[:], xts, INV2PI)
                tc_t = x_pool.tile([P, B], f16, name=f"tc{sub}")
                nc.vector.tensor_scalar_add(tc_t[:], ts_t[:], 0.25)
                kc_t = x_pool.tile([P, B], f16, name=f"kc{sub}")
                nc.vector.tensor_scalar(kc_t[:], tc_t[:], MAGIC, MAGIC, add_op, sub_op)
                rc_t = x_pool.tile([P, B], f16, name=f"rc{sub}")
                nc.vector.tensor_sub(rc_t[:], tc_t[:], kc_t[:])
                cosv = x_pool.tile([P, B], f16, name=f"cos{sub}")
                nc.scalar.activation(cosv[:], rc_t[:], Sin, bias=zero_h[:], scale=TWO_PI)
                coss.append(cosv)
                ks_t = x_pool.tile([P, B], f16, name=f"ks{sub}")
                nc.vector.tensor_scalar(ks_t[:], ts_t[:], MAGIC, MAGIC, add_op, sub_op)
                rs_t = x_pool.tile([P, B], f16, name=f"rs{sub}")
                nc.vector.tensor_sub(rs_t[:], ts_t[:], ks_t[:])
                sinv = x_pool.tile([P, B], f16, name=f"sin{sub}")
                nc.scalar.activation(sinv[:], rs_t[:], Sin, bias=zero_h[:], scale=TWO_PI)
                sins.append(sinv)

            out_ps = psum_out_pool.tile([NCLS, B], f32, name="out_ps")

            def finish_tensor(ti, vals, start):
                """Transpose row ti to per-partition cols, scale the
                SMALL fwt tiles by them (fwt[d,c]*S[d]), and accumulate
                (fwt*S).T @ trig into out_ps."""
                for sub in range(NSUB):
                    row_sb = const_pool.tile(
                        [1, P], f32, name=f"row{ti}_{sub}", tag=f"row{ti}_{sub}"
                    )
                    nc.vector.tensor_copy(
                        row_sb[:], rows[ti][0:1, sub * P : (sub + 1) * P]
                    )
                    ps = psum_pool.tile([P, 1], f32, name=f"ps{ti}_{sub}", tag="ps")
                    nc.tensor.matmul(
                        ps[:], row_sb[:], one1[:], start=True, stop=True
                    )
                    fws = x_pool.tile(
                        [P, NCLS], f16, name=f"fws{ti}_{sub}", tag=f"fws{ti}{sub}"
                    )
                    nc.vector.tensor_scalar_mul(fws[:], fwt[:, sub, :], ps[:])
                    nc.tensor.matmul(
                        out_ps[:],
                        fws[:],
                        vals[sub][:],
                        start=(start and sub == 0),
                        stop=(not start and sub == NSUB - 1),
                    )

            # a finishes mid-stream: its cos-side output matmuls overlap
            # the b stream; b's sin side forms the (short) tail.
            finish_tensor(0, coss, start=True)
            chunk_mms(1, b_hi, 0, 5)
            chunk_mms(1, b_lo, 0, 7)
            chunk_mms(1, b_hi, 5, 9)
            chunk_mms(1, b_lo, 7, 11)
            chunk_mms(1, b_lo, 11, 13)
            chunk_mms(1, b_hi, 9, 11)
            finish_tensor(1, sins, start=False)

            # ---------- output store ----------
            out_sb = const_pool.tile([NCLS, B], f32, name="out_sb")
            H = B // 2
            nc.scalar.copy(out_sb[:, 0:H], out_ps[:, 0:H])
            nc.vector.tensor_copy(out_sb[:, H:B], out_ps[:, H:B])
            PH = NCLS // 2
            nc.sync.dma_start(out=out_cb[0:PH, :], in_=out_sb[0:PH, :])
            nc.scalar.dma_start(out=out_cb[PH:NCLS, :], in_=out_sb[PH:NCLS, :])

    nc.compile()
    return nc


def _get_nc():
    if "nc" not in _STATE:
        _STATE["nc"] = _build()
    return _STATE["nc"]


def _pack_ab(t2, sl):
    """[D, DW] f32 slice -> [P, NCH*DW] fp16, chunk-major free dim."""
    s = t2[:, sl].reshape(NCH, P, DW).transpose(1, 0, 2).reshape(P, NCH * DW)
    return np.ascontiguousarray(s.astype(np.float16))


def _prep_in_maps(x, a, b, fc_w):
    xf = np.asarray(x, dtype=np.float32).reshape(B, D)
    xt = xf.T.astype(np.float16)  # [D, B]
    a2 = np.asarray(a, dtype=np.float32).reshape(D, D)
    b2 = np.asarray(b, dtype=np.float32).reshape(D, D)
    fw = np.asarray(fc_w, dtype=np.float32)
    in_maps = []
    for m in range(NCORES):
        sl = slice(m * DW, (m + 1) * DW)
        # combo [P, NSUB, B+NCLS]: xt part + fwt part per sub
        xs = xt[sl, :].reshape(NSUB, P, B)
        fs = fw[:, sl].T.reshape(NSUB, P, NCLS).astype(np.float16)
        cmb = np.concatenate([xs, fs], axis=2)          # [NSUB, P, B+NCLS]
        cmb = np.ascontiguousarray(
            cmb.transpose(1, 0, 2).reshape(P, NSUB * (B + NCLS))
        )
        in_maps.append(
            {
                "a_s": _pack_ab(a2, sl),
                "b_s": _pack_ab(b2, sl),
                "cmb_s": cmb,
            }
        )
    return in_maps


def _run(inputs, trace=False, trace_kwargs=None):
    """Run the device kernel; returns (final_output, BassKernelResults)."""
    from concourse.bass_utils import run_bass_kernel_spmd

    x = inputs["x"]
    a = inputs["a"]
    b = inputs["b"]
    w = np.asarray(inputs["w"], dtype=np.float64)
    n_param = np.asarray(inputs["n_param"], dtype=np.float64)
    fc_w = np.asarray(inputs["fc_w"], dtype=np.float32)
    fc_b = np.asarray(inputs["fc_b"], dtype=np.float32)

    nc = _get_nc()
    in_maps = _prep_in_maps(x, a, b, fc_w)
    res = run_bass_kernel_spmd(
        nc,
        in_maps,
        list(range(NCORES)),
        trace=trace,
        **(trace_kwargs or {}),
    )

    acc = np.zeros((NCLS, B), dtype=np.float32)
    for r in res.results:
        acc += r["out_cb"]
    part1 = float(np.sum(w[1:] * n_param[1:] + w[:-1] * n_param[:-1]))
    final = acc.T + np.float32(part1) * fc_w.sum(axis=1)[None, :] + fc_b[None, :]
    return np.ascontiguousarray(final.astype(np.float32)), res


def kernel(**inputs) -> np.ndarray:
    out, _ = _run(inputs, trace=False)
    return out
